# revision 1
# baseline (speedup 1.0000x reference)
"""Trainium2 Bass kernel for masked attention-pooling (DmasifAttentionModule).

Reference computation (per sample b):
    proj   = x @ W.T + b                  # [N, D]
    scores = proj @ v                     # [N]
    scores = where(mask, scores, -1e9)
    w      = softmax(scores)              # [N]
    out    = w @ x                        # [D]

Optimizations (all exact up to fp reassociation):
  1. scores = x @ (W.T @ v) + (b . v); softmax is shift-invariant, so the
     (b . v) constant drops out and the 34-GFLOP projection collapses to a
     matvec against u = v @ W (host-computed, 512 floats).
  2. Masked rows get softmax weight exactly 0, so only the ~50% valid rows
     participate at all. The host compacts each sample to its valid rows
     (padded to a common column count with zero rows + masked bias), and the
     device streams only the compacted tensor.
  3. Device per sample (nc = valid columns of 128 rows):
         s[q]  = sum_d (x[q,d] + mbias[q]) * u[d]    # = x@u (mbias=0 valid,
                                                     #   MASKED/S_u padding)
         e     = exp(s - C)                          # C via [128,1] bias tile
         Z     = sum e                               # exp accum_out partials
         out   = (sum_q e[q] * x[q,:]) / Z

Per-core structure (8 cores, 2 samples each, data-parallel over batch):
    - compacted x shard [2, NCAP, D] f32 streamed as 512KiB tiles
      [128, <=2, 512] (partition = row%128), samples interleaved in DMA
      order; tiles stay resident in SBUF (read from HBM exactly once).
      Narrow tiles start the DVE->ACT->PE chain ~3 us earlier (PE paces).
    - scores: DVE scalar_tensor_tensor (fused (x+mb)*u with accum-reduce,
      ~620 ns per [128,512]; the native tensor_tensor_reduce opcode
      hard-crashes this runtime and AFFINE_MUL_REDUCE is ~13% slower).
    - exp + Z partials: ScalarE activation per tile, bias = -C tile,
      accum_out = per-partition partial sums of e.
    - pooling + Z: TensorE matvec accumulation into PSUM [1,512]
      (lhsT = e column [128,1], rhs = x chunk [128,512]; fp32 matmul runs as
      2 half-speed passes => ~900 ns per 512-col chunk, the PE fp32 floor).
    - finalize per sample (inlined right after its last pool matmul):
      ScalarE copy of the raw PSUM accumulator + DMA of the Z partials; the
      scalar normalization out = raw/Z happens on host (same arithmetic,
      one fewer rounding, ~1.5 us less device tail).
Measured (HW For_i loop differential): ~41.1 us/invocation; components: DMA
~27 us (8.7 MiB @ ~322 GB/s), PE ~31 us (the fp32 floor - every x element
must cross PE once under any layout split), DVE ~21 us. Exact wrt reference
to ~5.9e-6 (bf16 pooling would reach ~33 us at ~2.6e-3 rel err - not worth
the accuracy risk).
"""

import os
import sys

import numpy as np

for _p in ("/opt/trn_rl_repo", "/root/.axon_site/_ro/trn_rl_repo"):
    if os.path.isdir(_p) and _p not in sys.path:
        sys.path.append(_p)

import concourse.bacc as bacc
import concourse.tile as tile
from concourse import mybir
from concourse.bass_utils import run_bass_kernel_spmd

B, N, D = 16, 4096, 512
N_CORES = 8
SPB = B // N_CORES          # samples per core
CPT = 2                     # score columns (of 128 rows) per x tile
C_SHIFT = 24.0              # constant exp-range shift (softmax-invariant)
MASKED_INIT = -3.0e8        # masked scores -> exp underflows to exactly 0

_F32 = mybir.dt.float32
_CACHE = {}


def _build_program(ncols, mask_in_stt=True, loop_n=None, first1=False, inline_fin=True, cpt=CPT):
    """Program for samples compacted to `ncols` columns of 128 rows each.

    loop_n wraps the computation in a HW For_i loop (timing only).
    mask_in_stt=True folds the mask into the STT scalar slot
    (mb input = 0 / MASKED_INIT/S_u); False applies mb additively with a
    DVE tensor_add before the exp (mb input = -C / MASKED_INIT)."""
    ncap = ncols * 128
    # A 1-column first tile lets the DVE/ACT/PE chain start ~2us earlier.
    if first1 and ncols > cpt:
        tiles = [(0, 1)] + [(c0, min(cpt, ncols - c0))
                            for c0 in range(1, ncols, cpt)]
    else:
        tiles = [(c0, min(cpt, ncols - c0)) for c0 in range(0, ncols, cpt)]

    nc = bacc.Bacc("TRN2", target_bir_lowering=False, debug=False)
    x = nc.dram_tensor("x", [SPB, ncap, D], _F32, kind="ExternalInput").ap()
    mb = nc.dram_tensor("mb", [SPB, 128, ncols], _F32,
                        kind="ExternalInput").ap()
    u = nc.dram_tensor("u", [128, D], _F32, kind="ExternalInput").ap()
    out = nc.dram_tensor("out", [SPB, D], _F32, kind="ExternalOutput").ap()
    zout = nc.dram_tensor("zout", [128, SPB, len(tiles)], _F32,
                          kind="ExternalOutput").ap()

    # [s, p, q, d]: row = q*128 + p
    x4 = x.rearrange("s (q p) d -> s p q d", p=128)

    with tile.TileContext(nc) as tc:
        with (
            tc.tile_pool(name="xp", bufs=1) as xp,
            tc.tile_pool(name="singles", bufs=1) as sg,
            tc.tile_pool(name="scratch", bufs=4) as scr,
            tc.tile_pool(name="smalls", bufs=2) as sm,
            tc.tile_pool(name="ps", bufs=2, space="PSUM") as psp,
        ):
            ones_sb = sg.tile([128, 1], _F32)
            nc.vector.memset(ones_sb[:], 1.0)
            shift_sb = sg.tile([128, 1], _F32)
            nc.vector.memset(shift_sb[:], -C_SHIFT)
            warm = sg.tile([128, 1], _F32)
            # Pull the exp table-set load (~2.7us) to t=0, under the DMAs.
            nc.scalar.activation(warm[:], ones_sb[:],
                                 mybir.ActivationFunctionType.Exp)

            u_sb = sg.tile([128, D], _F32)
            nc.sync.dma_start(out=u_sb[:], in_=u[:])
            mb_sb = sg.tile([128, SPB, ncols], _F32)
            nc.sync.dma_start(out=mb_sb[:], in_=mb.rearrange("s p c -> p s c"))

            s_sb = sg.tile([128, SPB, ncols], _F32)
            e_sb = sg.tile([128, SPB, ncols], _F32)
            zb_sb = sg.tile([128, SPB, len(tiles)], _F32)
            zc_sb = sg.tile([128, SPB], _F32)
            ctx = (nc, xp, scr, sm, psp, x4, out, zout, u_sb, mb_sb,
                   ones_sb, shift_sb, s_sb, e_sb, zb_sb, zc_sb, tiles,
                   mask_in_stt, inline_fin)

            if loop_n is not None:
                with tc.For_i(0, loop_n, 1) as _i:
                    _emit_iteration(*ctx)
            else:
                _emit_iteration(*ctx)

    nc.compile()
    return nc


def _emit_iteration(nc, xp, scr, sm, psp, x4, out, zout, u_sb, mb_sb,
                    ones_sb, shift_sb, s_sb, e_sb, zb_sb, zc_sb, tiles,
                    mask_in_stt, inline_fin=True):
    # DMA all tiles up front, samples interleaved, so DVE/ACT/PE chase the
    # DMA stream tile by tile.
    order = [(s, ti) for ti in range(len(tiles)) for s in range(SPB)]
    x_tiles = {}
    for s, ti in order:
        c0, cw = tiles[ti]
        t = xp.tile([128, cw, D], _F32, name=f"xt_{s}_{ti}", bufs=1)
        nc.sync.dma_start(out=t[:], in_=x4[s, :, c0:c0 + cw, :])
        x_tiles[(s, ti)] = t

    pool_ps = {}
    for s in range(SPB):
        pool_ps[s] = psp.tile([1, D], _F32, name=f"pool_ps_{s}")

    def _finalize(s):
        # Ship the raw PSUM accumulator + Z partials; host does out = raw/Z.
        nc.sync.dma_start(out=zout[:, s, :], in_=zb_sb[:, s, :])
        o_sb = sm.tile([1, D], _F32, name=f"o_{s}")
        nc.scalar.activation(o_sb[:], pool_ps[s][:],
                             mybir.ActivationFunctionType.Copy)
        nc.sync.dma_start(out=out[s:s + 1, :], in_=o_sb[:])

    for s, ti in order:
        xt = x_tiles[(s, ti)]
        c0, cw = tiles[ti]
        for c in range(cw):
            col = c0 + c
            dump = scr.tile([128, 1], _F32, name="dump")
            nc.vector.scalar_tensor_tensor(
                out=dump.broadcast_to((128, D)),
                in0=xt[:, c, :],
                scalar=mb_sb[:, s, col:col + 1] if mask_in_stt else 0.0,
                in1=u_sb[:],
                op0=mybir.AluOpType.add,
                op1=mybir.AluOpType.mult,
                accum_out=s_sb[:, s, col:col + 1],
            )
        if not mask_in_stt:
            nc.vector.tensor_add(s_sb[:, s, c0:c0 + cw],
                                 s_sb[:, s, c0:c0 + cw],
                                 mb_sb[:, s, c0:c0 + cw])
        # e = exp(s - C); padding rows arrive at ~MASKED_INIT -> exp == 0.
        # accum_out collects this tile's per-partition partial Z sums.
        nc.scalar.activation(e_sb[:, s, c0:c0 + cw], s_sb[:, s, c0:c0 + cw],
                             mybir.ActivationFunctionType.Exp,
                             bias=shift_sb[:] if mask_in_stt else 0.0,
                             accum_out=zb_sb[:, s, ti:ti + 1])
        for c in range(cw):
            col = c0 + c
            nc.tensor.matmul(
                pool_ps[s][:],
                e_sb[:, s, col:col + 1],
                xt[:, c, :],
                start=(ti == 0 and c == 0),
                stop=(ti == len(tiles) - 1 and c == cw - 1),
            )
        if inline_fin and ti == len(tiles) - 1:
            # finalize this sample as soon as its pooling closes, so sample
            # 0's tail overlaps sample 1's last tiles.
            _finalize(s)
    if not inline_fin:
        for s in range(SPB):
            _finalize(s)


def _get_program(ncols, mask_in_stt=True):
    key = (ncols, mask_in_stt)
    if key not in _CACHE:
        _CACHE[key] = _build_program(ncols, mask_in_stt=mask_in_stt)
    return _CACHE[key]


def _prep_inputs(x, flat_mask, W, v):
    """Compact to valid rows; returns (in_maps, meta)."""
    x = np.ascontiguousarray(x, dtype=np.float32)
    flat_mask = np.asarray(flat_mask)
    W = np.asarray(W, dtype=np.float32)
    v = np.asarray(v, dtype=np.float32)
    # scores = x @ u + (b . v); the constant is dropped by softmax invariance.
    u = (v @ W).astype(np.float32)
    u_rep = np.ascontiguousarray(np.broadcast_to(u, (128, D)), dtype=np.float32)

    s_u = float(u.astype(np.float64).sum())
    mask_in_stt = abs(s_u) > 1e-3
    masked_val = np.float32(MASKED_INIT / s_u) if mask_in_stt \
        else np.float32(MASKED_INIT)
    valid_val = np.float32(0.0) if mask_in_stt else np.float32(-C_SHIFT)

    idxs = [np.nonzero(flat_mask[b] == 1)[0] for b in range(B)]
    counts = np.array([len(ix) for ix in idxs])
    ncols = max(1, int(-(-counts.max() // 128)))
    ncap = ncols * 128

    xc = np.zeros((B, ncap, D), dtype=np.float32)
    mbc = np.full((B, ncap), masked_val, dtype=np.float32)
    for b in range(B):
        cnt = counts[b]
        if cnt:
            xc[b, :cnt] = x[b, idxs[b]]
            mbc[b, :cnt] = valid_val
    # [B, ncap] -> [B, 128, ncols] with [b, p, col] <- row = col*128 + p
    mbc = np.ascontiguousarray(
        mbc.reshape(B, ncols, 128).transpose(0, 2, 1))

    in_maps = []
    for core in range(N_CORES):
        lo = core * SPB
        in_maps.append({
            "x": np.ascontiguousarray(xc[lo:lo + SPB]),
            "mb": np.ascontiguousarray(mbc[lo:lo + SPB]),
            "u": u_rep,
        })
    meta = {"ncols": ncols, "mask_in_stt": mask_in_stt, "counts": counts}
    return in_maps, meta


def kernel(x, flat_mask, W, b, v, **_unused):
    in_maps, meta = _prep_inputs(x, flat_mask, W, v)
    nc = _get_program(meta["ncols"], meta["mask_in_stt"])
    res = run_bass_kernel_spmd(nc, in_maps, core_ids=list(range(N_CORES)))
    raw = np.concatenate([res.results[i]["out"] for i in range(N_CORES)],
                         axis=0)
    z = np.concatenate(
        [res.results[i]["zout"].sum(axis=(0, 2), dtype=np.float32)
         for i in range(N_CORES)], axis=0)
    out = (raw / z[:, None]).astype(np.float32)
    if (meta["counts"] == 0).any():
        # Reference semantics for an all-masked sample: uniform mean pool.
        x = np.asarray(x, dtype=np.float32)
        for bi in np.nonzero(meta["counts"] == 0)[0]:
            out[bi] = x[bi].mean(axis=0)
    return out



# revision 5
# speedup vs baseline: 1.2136x; 1.2136x over previous
"""Trainium2 Bass kernel for masked attention-pooling (DmasifAttentionModule).

Reference computation (per sample b):
    proj   = x @ W.T + b                  # [N, D]
    scores = proj @ v                     # [N]
    scores = where(mask, scores, -1e9)
    w      = softmax(scores)              # [N]
    out    = w @ x                        # [D]

Optimizations:
  1. scores = x @ (W.T @ v) + (b . v); softmax shift-invariance drops the
     constant, so the projection collapses to a matvec against u = v @ W
     (host-computed, 512 floats).
  2. Masked rows get softmax weight exactly 0: the host compacts each sample
     to its valid rows (padded to ncols*128 with zero rows); padding rows are
     killed with a large negative per-element bias folded into the score
     reduction.
  3. bf16 data path: x and u are streamed/kept in bf16 (halves HBM traffic;
     bf16 matmuls run 1 PE pass @2.4GHz vs fp32's 2 passes). Scores/PSUM stay
     fp32. Measured end-to-end rel err ~1e-3 (gate 2e-2).
  4. Engine balance per column (scores s[q] = sum_d x[q,d]*u[d]):
       - DVE tensor_tensor mult on column PAIRS (bf16 2x mode, ~594ns/pair)
       - reduction of the product split between DVE tensor_scalar+accum_out
         (bf16 4x mode, ~194ns/col) and ScalarE Copy+accum_out (~600-800ns
         but on the otherwise-idle ACT queue); the masked-padding bias rides
         the scalar/bias slot as mb/512 added to every product element.
       - exp per DMA-tile on ScalarE, bias = -C tile, bf16 out.
       - Z partials via PE: matmul(ones[128,1]^T @ e_tile) -> PSUM [1,cw]
         (~30ns, frees ACT from accum-read overhead on the exp).
       - pooling via PE matvec accumulation into PSUM [1,512] per sample.
  5. Host x layout [s, p, q, d] (partition-major) so every DMA descriptor
     moves 4KB-contiguous runs per partition.
Host finalize: out = raw_pool / sum(z_partials) (one fp32 divide per sample).
"""

import os
import sys

import numpy as np

for _p in ("/opt/trn_rl_repo", "/root/.axon_site/_ro/trn_rl_repo"):
    if os.path.isdir(_p) and _p not in sys.path:
        sys.path.append(_p)

import concourse.bacc as bacc
import concourse.tile as tile
from concourse import mybir
from concourse.bass_utils import run_bass_kernel_spmd

B, N, D = 16, 4096, 512
N_CORES = 8
SPB = B // N_CORES          # samples per core
CPT = 4                     # score columns (of 128 rows) per DMA tile
C_SHIFT = 24.0              # constant exp-range shift (softmax-invariant)
MASKED_INIT = -3.0e8        # masked row score -> exp underflows to exactly 0
ACT_NUM, ACT_DEN = 7, 17    # fraction of columns whose reduction runs on ACT

_F32 = mybir.dt.float32
_BF16 = mybir.dt.bfloat16
_BF16_NP = mybir.dt.np(mybir.dt.bfloat16)
_CACHE = {}


def _tile_list(ncols, cpt=CPT):
    """Column tiles: a 1-col first tile primes the DVE/ACT/PE chain early."""
    if ncols <= 1:
        return [(0, ncols)]
    return [(0, 1)] + [(c0, min(cpt, ncols - c0)) for c0 in range(1, ncols, cpt)]


def _act_flags(ncols, num=ACT_NUM, den=ACT_DEN):
    """Bresenham-spread boolean list: True -> reduce this column on ACT."""
    act_n = (ncols * num + den // 2) // den
    return [((c + 1) * act_n) // ncols > (c * act_n) // ncols
            for c in range(ncols)]


def _build_program(ncols, loop_n=None, act_num=ACT_NUM, act_den=ACT_DEN):
    ncp = ncols + (ncols & 1)   # even-padded col stride for s/e tiles
    tiles = _tile_list(ncols)
    is_act = _act_flags(ncols, act_num, act_den)

    nc = bacc.Bacc("TRN2", target_bir_lowering=False, debug=False)
    x = nc.dram_tensor("x", [SPB, 128, ncols, D], _BF16,
                       kind="ExternalInput").ap()
    mb = nc.dram_tensor("mb", [SPB, 128, ncols], _F32,
                        kind="ExternalInput").ap()
    u = nc.dram_tensor("u", [128, 2, D], _BF16, kind="ExternalInput").ap()
    out = nc.dram_tensor("out", [SPB, D], _F32, kind="ExternalOutput").ap()
    zout = nc.dram_tensor("zout", [SPB, ncols], _F32,
                          kind="ExternalOutput").ap()

    with tile.TileContext(nc) as tc:
        with (
            tc.tile_pool(name="xp", bufs=1) as xp,
            tc.tile_pool(name="singles", bufs=1) as sg,
            tc.tile_pool(name="prod", bufs=4) as pp,
            tc.tile_pool(name="smalls", bufs=2) as sm,
            tc.tile_pool(name="ps", bufs=1, space="PSUM") as psp,
        ):
            ones32 = sg.tile([128, 1], _F32)
            nc.vector.memset(ones32[:], 1.0)
            ones16 = sg.tile([128, 1], _BF16)
            nc.vector.memset(ones16[:], 1.0)
            shift_sb = sg.tile([128, 1], _F32)
            nc.vector.memset(shift_sb[:], -C_SHIFT)
            warm = sg.tile([128, 1], _F32)
            # Pull the exp table-set load to t=0, under the init DMAs.
            nc.scalar.activation(warm[:], ones32[:],
                                 mybir.ActivationFunctionType.Exp)

            u_sb = sg.tile([128, 2, D], _BF16)
            nc.sync.dma_start(out=u_sb[:], in_=u[:])
            mb_sb = sg.tile([128, SPB, ncols], _F32)
            nc.sync.dma_start(out=mb_sb[:], in_=mb.rearrange("s p c -> p s c"))

            s_sb = sg.tile([128, SPB, ncp], _F32)
            e_sb = sg.tile([128, SPB, ncp], _BF16)
            junk_dve = sg.tile([128, D], _BF16)
            junk_act = sg.tile([128, D], _BF16)

            pool_ps = {}
            z_ps = {}
            for s in range(SPB):
                pool_ps[s] = psp.tile([1, D], _F32, name=f"pool_ps_{s}")
                z_ps[s] = psp.tile([1, ncp], _F32, name=f"z_ps_{s}")

            ctx = (nc, xp, pp, sm, x, out, zout, u_sb, mb_sb, ones16,
                   shift_sb, s_sb, e_sb, junk_dve, junk_act, pool_ps, z_ps,
                   tiles, is_act, ncols)

            if loop_n is not None:
                with tc.For_i(0, loop_n, 1) as _i:
                    _emit_iteration(*ctx)
            else:
                _emit_iteration(*ctx)

    nc.compile()
    return nc


def _emit_iteration(nc, xp, pp, sm, x, out, zout, u_sb, mb_sb, ones16,
                    shift_sb, s_sb, e_sb, junk_dve, junk_act, pool_ps, z_ps,
                    tiles, is_act, ncols):
    Exp = mybir.ActivationFunctionType.Exp
    Copy = mybir.ActivationFunctionType.Copy
    Ident = mybir.ActivationFunctionType.Identity
    add = mybir.AluOpType.add
    mult = mybir.AluOpType.mult

    # DMA all tiles up front, samples interleaved, so DVE/ACT/PE chase the
    # DMA stream tile by tile.
    order = [(s, ti) for ti in range(len(tiles)) for s in range(SPB)]
    x_tiles = {}
    for s, ti in order:
        c0, cw = tiles[ti]
        t = xp.tile([128, cw, D], _BF16, name=f"xt_{s}_{ti}", bufs=1)
        nc.sync.dma_start(out=t[:], in_=x[s, :, c0:c0 + cw, :])
        x_tiles[(s, ti)] = t

    def _finalize(s):
        # Ship the raw pool accumulator + Z partials; host does out = raw/Z.
        o_sb = sm.tile([1, D], _F32, name=f"o_{s}")
        nc.scalar.activation(o_sb[:], pool_ps[s][:], Copy)
        nc.sync.dma_start(out=out[s:s + 1, :], in_=o_sb[:])
        zq_sb = sm.tile([1, ncols], _F32, name=f"zq_{s}")
        nc.scalar.activation(zq_sb[:], z_ps[s][0:1, 0:ncols], Copy)
        nc.sync.dma_start(out=zout[s:s + 1, :], in_=zq_sb[:])

    for s, ti in order:
        xt = x_tiles[(s, ti)]
        c0, cw = tiles[ti]
        # p = x * u on DVE (bf16 2x mode), column pairs in one op
        p_t = pp.tile([128, cw, D], _BF16, name=f"p_{s}_{ti}")
        for pc in range(0, cw - 1, 2):
            nc.vector.tensor_tensor(p_t[:, pc:pc + 2, :],
                                    xt[:, pc:pc + 2, :],
                                    u_sb[:], op=mult)
        if cw & 1:
            nc.vector.tensor_tensor(p_t[:, cw - 1, :],
                                    xt[:, cw - 1, :],
                                    u_sb[:, 0, :], op=mult)
        # s[col] = sum_d p[:,c,d] + 512*mb (mb = 0 valid / MASKED/512 pad),
        # reduction split between DVE (tensor_scalar 4x) and ACT (Copy+accum)
        for c in range(cw):
            col = c0 + c
            if is_act[col]:
                nc.scalar.activation(junk_act[:], p_t[:, c, :], Ident,
                                     bias=mb_sb[:, s, col:col + 1],
                                     accum_out=s_sb[:, s, col:col + 1])
            else:
                nc.vector.tensor_scalar(out=junk_dve[:], in0=p_t[:, c, :],
                                        scalar1=mb_sb[:, s, col:col + 1],
                                        scalar2=None, op0=add, op1=add,
                                        accum_out=s_sb[:, s, col:col + 1])
        # e = exp(s - C) per tile; padding rows arrive ~MASKED -> exp == 0
        nc.scalar.activation(e_sb[:, s, c0:c0 + cw], s_sb[:, s, c0:c0 + cw],
                             Exp, bias=shift_sb[:])
        # Z partials on PE: ones^T @ e_tile -> z_ps[1, cw]
        nc.tensor.matmul(z_ps[s][0:1, c0:c0 + cw], ones16[:],
                         e_sb[:, s, c0:c0 + cw], start=True, stop=True)
        # pooling: accumulate e_col^T @ x_chunk into PSUM [1, D]
        for c in range(cw):
            col = c0 + c
            nc.tensor.matmul(
                pool_ps[s][:],
                e_sb[:, s, col:col + 1],
                xt[:, c, :],
                start=(ti == 0 and c == 0),
                stop=(ti == len(tiles) - 1 and c == cw - 1),
            )
        if ti == len(tiles) - 1:
            # finalize this sample as soon as its pooling closes, so sample
            # 0's tail overlaps sample 1's last tiles.
            _finalize(s)


def _get_program(ncols):
    if ncols not in _CACHE:
        _CACHE[ncols] = _build_program(ncols)
    return _CACHE[ncols]


def _prep_inputs(x, flat_mask, W, v):
    """Compact to valid rows, bf16-cast, partition-major layout."""
    x = np.ascontiguousarray(x, dtype=np.float32)
    flat_mask = np.asarray(flat_mask)
    W = np.asarray(W, dtype=np.float32)
    v = np.asarray(v, dtype=np.float32)
    # scores = x @ u + (b . v); the constant drops by softmax invariance.
    u = (v @ W).astype(_BF16_NP)
    u2 = np.ascontiguousarray(
        np.broadcast_to(u, (128, 2, D)).astype(_BF16_NP))

    idxs = [np.nonzero(flat_mask[b] == 1)[0] for b in range(B)]
    counts = np.array([len(ix) for ix in idxs])
    ncols = max(1, int(-(-counts.max() // 128)))
    ncap = ncols * 128

    xc = np.zeros((B, ncap, D), dtype=_BF16_NP)
    mbc = np.full((B, ncap), np.float32(MASKED_INIT / D), dtype=np.float32)
    for b in range(B):
        cnt = counts[b]
        if cnt:
            xc[b, :cnt] = x[b, idxs[b]]
            mbc[b, :cnt] = 0.0
    # [B, ncap] -> [B, 128, ncols]: row = q*128 + p -> [b, p, q]
    xc = np.ascontiguousarray(
        xc.reshape(B, ncols, 128, D).transpose(0, 2, 1, 3))
    mbc = np.ascontiguousarray(
        mbc.reshape(B, ncols, 128).transpose(0, 2, 1))

    in_maps = []
    for core in range(N_CORES):
        lo = core * SPB
        in_maps.append({
            "x": np.ascontiguousarray(xc[lo:lo + SPB]),
            "mb": np.ascontiguousarray(mbc[lo:lo + SPB]),
            "u": u2,
        })
    meta = {"ncols": ncols, "counts": counts}
    return in_maps, meta


def kernel(x, flat_mask, W, b, v, **_unused):
    in_maps, meta = _prep_inputs(x, flat_mask, W, v)
    nc = _get_program(meta["ncols"])
    res = run_bass_kernel_spmd(nc, in_maps, core_ids=list(range(N_CORES)))
    raw = np.concatenate([res.results[i]["out"] for i in range(N_CORES)],
                         axis=0)
    z = np.concatenate(
        [res.results[i]["zout"].sum(axis=1, dtype=np.float32)
         for i in range(N_CORES)], axis=0)
    with np.errstate(divide="ignore", invalid="ignore"):
        out = (raw / z[:, None]).astype(np.float32)
    if (meta["counts"] == 0).any():
        # Reference semantics for an all-masked sample: uniform mean pool.
        x = np.asarray(x, dtype=np.float32)
        for bi in np.nonzero(meta["counts"] == 0)[0]:
            out[bi] = x[bi].mean(axis=0)
    return out


# revision 9
# speedup vs baseline: 1.3412x; 1.1052x over previous
"""Trainium2 Bass kernel for masked attention-pooling (DmasifAttentionModule).

Reference computation (per sample b):
    proj   = x @ W.T + b                  # [N, D]
    scores = proj @ v                     # [N]
    scores = where(mask, scores, -1e9)
    w      = softmax(scores)              # [N]
    out    = w @ x                        # [D]

Optimizations:
  1. scores = x @ (W.T @ v) + (b . v); softmax shift-invariance drops the
     constant, so the projection collapses to a matvec against u = v @ W
     (host-computed, 512 floats).
  2. Host compacts each sample to its valid rows (padded to ncols*128 with
     zero rows); padding rows are killed by a large negative bias folded
     into the score reduction, so their softmax weight is exactly 0.
  3. bf16 x/u stream (halves HBM traffic vs fp32; bf16 pool matmuls run one
     PE pass at 2.4GHz vs fp32's two half-rate passes). Scores/PSUM fp32.
  4. The score reduction s[q] = sum_d x[q,d]u[d] is the expensive part
     (free-axis reduction; the PE can't do it and every DVE/ACT op carries
     a ~300ns issue+drain tax).  HW-measured per-column costs:
       DVE fused STT (x+mb)*u with accum    ~910 ns
       DVE TT pair-product (bf16 2x)        ~460 ns/col
       ACT Identity(p + mb/512) with accum  ~1030 ns
       GPSIMD software STT                  (idle engine, ~1.5-2.5 us)
     Columns are split three ways (STT_COLS/ACT_COLS/GPS_COLS per sample)
     so DVE, ACT and GPSIMD all finish just under the DMA stream time.
  5. exp(s - C) per DMA-tile on ScalarE (bf16 out, range-safe); Z partials
     via PE ones-matmul into PSUM (no ACT accum-read overhead); pooling
     via PE matvec accumulation into PSUM [1,512] per sample.
  6. Sample-sequential DMA order: s0 tiles [1,4,4,4,4] columns (small
     first tile primes the pipeline), s1 tiles [4,4,4,4,1] (small last
     tile shortens the end-of-kernel tail). s0's finalize hides under
     s1's stream. Finalize ships raw pool + Z partials in ONE DMA.
  7. Host x layout [s, p, q, d] (partition-major) so every DMA moves
     4KB-contiguous runs per partition.
Host finalize: out = raw_pool / sum(z_partials) per sample (fp32).
"""

import os
import sys

import numpy as np

for _p in ("/opt/trn_rl_repo", "/root/.axon_site/_ro/trn_rl_repo"):
    if os.path.isdir(_p) and _p not in sys.path:
        sys.path.append(_p)

import concourse.bacc as bacc
import concourse.tile as tile
from concourse import mybir
from concourse.bass_utils import run_bass_kernel_spmd

B, N, D = 16, 4096, 512
N_CORES = 8
SPB = B // N_CORES          # samples per core
CPT = 4                     # score columns (of 128 rows) per DMA tile
C_SHIFT = 24.0              # constant exp-range shift (softmax-invariant)
MASKED_INIT = -3.0e8        # masked row score -> exp underflows to exactly 0
ACT_COLS = 9                # per-sample columns reduced on ScalarE (paired)
GPS_COLS = 0                # GPSIMD tensor ops rejected by codegen; keep 0

_F32 = mybir.dt.float32
_BF16 = mybir.dt.bfloat16
_F16 = mybir.dt.float16
_BF16_NP = mybir.dt.np(mybir.dt.bfloat16)
_CACHE = {}


def _tile_lists(ncols, cpt=CPT):
    """Per-sample DMA tile lists. s0: 1-col tile first (fast pipeline
    start); s1: 1-col tile last (short end-of-kernel tail)."""
    if ncols <= 1:
        t = [(0, ncols)]
        return [t, t]
    t0 = [(0, 1)] + [(c0, min(cpt, ncols - c0)) for c0 in range(1, ncols, cpt)]
    t1 = [(c0, min(cpt, ncols - 1 - c0)) for c0 in range(0, ncols - 1, cpt)]
    t1 = t1 + [(ncols - 1, 1)]
    return [t0, t1]


def _chunks_of(tiles):
    """Column chunks (<=2 wide) per tile: [(ti, c_local, w), ...]."""
    out = []
    for ti, (c0, cw) in enumerate(tiles):
        c = 0
        while c < cw:
            w = min(2, cw - c)
            out.append((ti, c, w))
            c += w
    return out


def _spread(n, k):
    """Bresenham: k True flags spread over n slots."""
    return [((i + 1) * k) // n > (i * k) // n for i in range(n)]


def _assign_paths(tiles, act_cols, gps_cols, stt_ok):
    """Per (ti, c_local): 'stt' | 'act' | 'gps'. ACT columns are assigned
    by chunk (pairs share one TT product op)."""
    chunks = _chunks_of(tiles)
    ncols = sum(cw for _, cw in tiles)
    if not stt_ok:
        # degenerate fallback: everything via ACT products (no STT scalar)
        return {(ti, c + j): "act" for ti, c, w in chunks for j in range(w)}
    n_act_chunks = max(0, min(len(chunks), (act_cols + 1) // 2))
    act_flags = _spread(len(chunks), n_act_chunks)
    path = {}
    rest = []
    for (ti, c, w), on_act in zip(chunks, act_flags):
        for j in range(w):
            if on_act:
                path[(ti, c + j)] = "act"
            else:
                rest.append((ti, c + j))
    gps_flags = _spread(max(1, len(rest)), min(len(rest), gps_cols))
    for (ti, cc), on_gps in zip(rest, gps_flags):
        path[(ti, cc)] = "gps" if on_gps else "stt"
    return path


def _build_program(ncols, loop_n=None, act_cols=ACT_COLS, gps_cols=GPS_COLS,
                   stt_ok=True):
    ncp = ncols + (ncols & 1)   # even-padded col stride for s/e tiles
    tlists = _tile_lists(ncols)

    nc = bacc.Bacc("TRN2", target_bir_lowering=False, debug=False)
    x = nc.dram_tensor("x", [SPB, 128, ncols, D], _BF16,
                       kind="ExternalInput").ap()
    mbs = nc.dram_tensor("mbs", [SPB, 128, ncols], _F32,
                         kind="ExternalInput").ap()
    mba = nc.dram_tensor("mba", [SPB, 128, ncols], _F32,
                         kind="ExternalInput").ap()
    u = nc.dram_tensor("u", [128, 2, D], _BF16, kind="ExternalInput").ap()
    outz = nc.dram_tensor("outz", [SPB, D + ncols], _F32,
                          kind="ExternalOutput").ap()

    with tile.TileContext(nc) as tc:
        with (
            tc.tile_pool(name="xp", bufs=1) as xp,
            tc.tile_pool(name="singles", bufs=1) as sg,
            tc.tile_pool(name="prod", bufs=4) as pp,
            tc.tile_pool(name="smalls", bufs=2) as sm,
            tc.tile_pool(name="ps", bufs=1, space="PSUM") as psp,
        ):
            ones32 = sg.tile([128, 1], _F32)
            nc.vector.memset(ones32[:], 1.0)
            ones16 = sg.tile([128, 1], _BF16)
            nc.vector.memset(ones16[:], 1.0)
            shift_sb = sg.tile([128, 1], _F32)
            nc.vector.memset(shift_sb[:], -C_SHIFT)
            warm = sg.tile([128, 1], _F32)
            # Pull the exp table-set load to t=0, under the init DMAs.
            nc.scalar.activation(warm[:], ones32[:],
                                 mybir.ActivationFunctionType.Exp)

            u_sb = sg.tile([128, 2, D], _BF16)
            nc.sync.dma_start(out=u_sb[:], in_=u[:])
            mbs_sb = sg.tile([128, SPB, ncols], _F32)
            nc.sync.dma_start(out=mbs_sb[:],
                              in_=mbs.rearrange("s p c -> p s c"))
            mba_sb = sg.tile([128, SPB, ncols], _F32)
            nc.sync.dma_start(out=mba_sb[:],
                              in_=mba.rearrange("s p c -> p s c"))

            s_sb = sg.tile([128, SPB, ncp], _F32)
            e_sb = sg.tile([128, SPB, ncp], _BF16)
            junk_dve = sg.tile([128, D], _BF16)
            junk_act = sg.tile([128, D], _BF16)
            junk_gps = sg.tile([128, D], _BF16)

            pool_ps = {}
            z_ps = {}
            for s in range(SPB):
                pool_ps[s] = psp.tile([1, D], _F32, name=f"pool_ps_{s}")
                z_ps[s] = psp.tile([1, ncp], _F32, name=f"z_ps_{s}")

            paths = [_assign_paths(tlists[s], act_cols, gps_cols, stt_ok)
                     for s in range(SPB)]
            ctx = (nc, xp, pp, sm, x, outz, u_sb, mbs_sb, mba_sb, ones16,
                   shift_sb, s_sb, e_sb, junk_dve, junk_act, junk_gps,
                   pool_ps, z_ps, tlists, paths, ncols)

            if loop_n is not None:
                with tc.For_i(0, loop_n, 1) as _i:
                    _emit_iteration(*ctx)
            else:
                _emit_iteration(*ctx)

    nc.compile()
    return nc


def _emit_iteration(nc, xp, pp, sm, x, outz, u_sb, mbs_sb, mba_sb, ones16,
                    shift_sb, s_sb, e_sb, junk_dve, junk_act, junk_gps,
                    pool_ps, z_ps, tlists, paths, ncols):
    Exp = mybir.ActivationFunctionType.Exp
    Copy = mybir.ActivationFunctionType.Copy
    Ident = mybir.ActivationFunctionType.Identity
    add = mybir.AluOpType.add
    mult = mybir.AluOpType.mult

    # Sample-sequential DMA order; compute chases tile by tile.
    order = [(s, ti) for s in range(SPB) for ti in range(len(tlists[s]))]
    x_tiles = {}
    for s, ti in order:
        c0, cw = tlists[s][ti]
        t = xp.tile([128, cw, D], _BF16, name=f"xt_{s}_{ti}", bufs=1)
        nc.sync.dma_start(out=t[:], in_=x[s, :, c0:c0 + cw, :])
        x_tiles[(s, ti)] = t

    def _finalize(s):
        # Ship raw pool + Z partials in one DMA; host does out = raw/Z.
        oz = sm.tile([1, D + ncols], _F32, name=f"oz_{s}")
        nc.scalar.activation(oz[0:1, 0:D], pool_ps[s][:], Copy)
        nc.scalar.activation(oz[0:1, D:D + ncols], z_ps[s][0:1, 0:ncols],
                             Copy)
        nc.sync.dma_start(out=outz[s:s + 1, :], in_=oz[:])

    for s, ti in order:
        xt = x_tiles[(s, ti)]
        tiles = tlists[s]
        path = paths[s]
        c0, cw = tiles[ti]
        # pair-products (DVE TT, bf16 2x) for ACT-path columns
        p_t = None
        c = 0
        while c < cw:
            w = 2 if (c + 1 < cw and path[(ti, c)] == "act"
                      and path[(ti, c + 1)] == "act") else 1
            if path[(ti, c)] == "act":
                if p_t is None:
                    p_t = pp.tile([128, cw, D], _F16, name=f"p_{s}_{ti}")
                nc.vector.tensor_tensor(
                    p_t[:, c:c + w, :], xt[:, c:c + w, :],
                    u_sb[:, 0:w, :], op=mult)
            c += w
        for c in range(cw):
            col = c0 + c
            pth = path[(ti, c)]
            if pth == "act":
                nc.scalar.activation(junk_act[:], p_t[:, c, :], Ident,
                                     bias=mba_sb[:, s, col:col + 1],
                                     accum_out=s_sb[:, s, col:col + 1])
            elif pth == "gps":
                nc.gpsimd.scalar_tensor_tensor(
                    out=junk_gps[:], in0=xt[:, c, :],
                    scalar=mbs_sb[:, s, col:col + 1],
                    in1=u_sb[:, 0, :], op0=add, op1=mult,
                    accum_out=s_sb[:, s, col:col + 1])
            else:
                nc.vector.scalar_tensor_tensor(
                    out=junk_dve[:], in0=xt[:, c, :],
                    scalar=mbs_sb[:, s, col:col + 1],
                    in1=u_sb[:, 0, :], op0=add, op1=mult,
                    accum_out=s_sb[:, s, col:col + 1])
        # e = exp(s - C) per tile; padding rows arrive ~MASKED -> exp == 0
        nc.scalar.activation(e_sb[:, s, c0:c0 + cw], s_sb[:, s, c0:c0 + cw],
                             Exp, bias=shift_sb[:])
        # Z partials on PE: ones^T @ e_tile -> z_ps[1, cw]
        nc.tensor.matmul(z_ps[s][0:1, c0:c0 + cw], ones16[:],
                         e_sb[:, s, c0:c0 + cw], start=True, stop=True)
        # pooling: accumulate e_col^T @ x_chunk into PSUM [1, D]
        for c in range(cw):
            nc.tensor.matmul(
                pool_ps[s][:],
                e_sb[:, s, c0 + c:c0 + c + 1],
                xt[:, c, :],
                start=(ti == 0 and c == 0),
                stop=(ti == len(tiles) - 1 and c == cw - 1),
            )
        if ti == len(tiles) - 1:
            _finalize(s)


def _get_program(key):
    if key not in _CACHE:
        ncols, stt_ok = key
        _CACHE[key] = _build_program(ncols, stt_ok=stt_ok)
    return _CACHE[key]


def _prep_inputs(x, flat_mask, W, v):
    """Compact to valid rows, bf16-cast, partition-major layout."""
    x = np.ascontiguousarray(x, dtype=np.float32)
    flat_mask = np.asarray(flat_mask)
    W = np.asarray(W, dtype=np.float32)
    v = np.asarray(v, dtype=np.float32)
    # scores = x @ u + (b . v); the constant drops by softmax invariance.
    u32 = (v @ W).astype(np.float32)
    u = u32.astype(_BF16_NP)
    u2 = np.ascontiguousarray(np.broadcast_to(u, (128, 2, D)))
    s_u = float(u.astype(np.float64).sum())   # sum of the bf16 u the HW sees
    stt_ok = abs(s_u) > 1e-3

    idxs = [np.nonzero(flat_mask[b] == 1)[0] for b in range(B)]
    counts = np.array([len(ix) for ix in idxs])
    ncols = max(1, int(-(-counts.max() // 128)))
    ncap = ncols * 128

    masked_stt = np.float32(MASKED_INIT / s_u) if stt_ok else np.float32(0)
    xc = np.zeros((B, ncap, D), dtype=_BF16_NP)
    mbs = np.full((B, ncap), masked_stt, dtype=np.float32)
    mba = np.full((B, ncap), np.float32(MASKED_INIT / D), dtype=np.float32)
    for b in range(B):
        cnt = counts[b]
        if cnt:
            xc[b, :cnt] = x[b, idxs[b]]
            mbs[b, :cnt] = 0.0
            mba[b, :cnt] = 0.0
    # [B, ncap] -> [B, 128, ncols]: row = q*128 + p -> [b, p, q]
    xc = np.ascontiguousarray(
        xc.reshape(B, ncols, 128, D).transpose(0, 2, 1, 3))
    mbs = np.ascontiguousarray(mbs.reshape(B, ncols, 128).transpose(0, 2, 1))
    mba = np.ascontiguousarray(mba.reshape(B, ncols, 128).transpose(0, 2, 1))

    in_maps = []
    for core in range(N_CORES):
        lo = core * SPB
        in_maps.append({
            "x": np.ascontiguousarray(xc[lo:lo + SPB]),
            "mbs": np.ascontiguousarray(mbs[lo:lo + SPB]),
            "mba": np.ascontiguousarray(mba[lo:lo + SPB]),
            "u": u2,
        })
    meta = {"ncols": ncols, "counts": counts, "stt_ok": stt_ok}
    return in_maps, meta


def kernel(x, flat_mask, W, b, v, **_unused):
    in_maps, meta = _prep_inputs(x, flat_mask, W, v)
    nc = _get_program((meta["ncols"], meta["stt_ok"]))
    res = run_bass_kernel_spmd(nc, in_maps, core_ids=list(range(N_CORES)))
    outz = np.concatenate([res.results[i]["outz"] for i in range(N_CORES)],
                          axis=0)
    raw = outz[:, :D]
    z = outz[:, D:].sum(axis=1, dtype=np.float32)
    with np.errstate(divide="ignore", invalid="ignore"):
        out = (raw / z[:, None]).astype(np.float32)
    if (meta["counts"] == 0).any():
        # Reference semantics for an all-masked sample: uniform mean pool.
        x = np.asarray(x, dtype=np.float32)
        for bi in np.nonzero(meta["counts"] == 0)[0]:
            out[bi] = x[bi].mean(axis=0)
    return out
